# revision 9
# baseline (speedup 1.0000x reference)
"""Trainium2 Bass kernel for nn_BottomUpNet (dense_mlp).

Reference computation (per row n of N=8192, fully independent across rows):
    summary = aggregate (broadcast)                   # (1024,)
    for k in 0..15:
        x = [summary, towers[n, k, :]]                # (1088,)
        h = relu(x @ OW1 + Ob1); h = relu(h @ OW2 + Ob2)
        pred_k = sigmoid(h @ OW3 + Ob3)
        m = relu(x @ MW1 + Mb1); m = relu(m @ MW2 + Mb2); m = relu(m @ MW3 + Mb3)
        summary = m
    out[n] = prod_k pred_k

Strategy: data-parallel over N across 8 cores (1024 rows each), weights
replicated.  Activations are feature-major ([feature partition, row free]).

fp8 DoubleRow: all five big matmuls (M1s/O1s summary parts, M2, M3, O2) run
in fp8e4 (e4m3) with MatmulPerfMode.DoubleRow -- the PE processes two
128-deep contraction blocks per pass, 2x the bf16 MAC rate.  Weights are
packed host-side into contraction pairs [4][128, 2, 1024] scaled by 2^12;
activations are stored as fp8 pair tiles [128, 2, 512] with per-tensor
power-of-2 scales (summary 2^8, m1 2^6, m2 2^7, h1 2^6).  The 64-deep tower
matmuls stay bf16 (2.4% of cycles) with their weights pre-scaled by
s_act*2^12 so each PSUM group accumulates in one consistent scale; the
single scalar-engine epilogue then applies relu(psum*k + bias*s_next) and
writes fp8 directly.  Expected accuracy ~7e-3 max rel err (numpy sim), well
under the 2e-2 gate; expected time ~1.15ms vs 2.25ms bf16 baseline.

Other structure (inherited from the bf16 baseline):
  - step 0's summary contribution is rank-1 (broadcast aggregate): computed
    once as v = agg @ W1s and injected per tile by a contraction-1 matmul
    against a ones row.
  - layer-1 tower matmuls for the M/O branches pair into disjoint PE row
    groups (0-63 / 64-127) so they stream concurrently.
  - the 1024->1 output head is a DVE per-partition multiply/add tree over
    bf16 h2 plus a ones-vector matmul for the cross-partition reduce; its
    sigmoid + product-accumulate are deferred into the next step.
  - the final step's M branch is dead (scan carry discarded) and skipped.
"""

import numpy as np
import ml_dtypes

import concourse.bacc as bacc
import concourse.mybir as mybir
import concourse.tile as tile
from concourse.bass import ts, ds
from concourse.bass_utils import run_bass_kernel_spmd

BF16 = ml_dtypes.bfloat16
FP8 = ml_dtypes.float8_e4m3

N_CORES = 8
N = 8192
K = 16
NI = 64          # tower features per step
NH = 1024        # hidden width
FT = NH // 128   # feature tiles (8)
NP = FT // 2     # contraction pairs (4)
R = N // N_CORES  # rows per core (1024)
RB = 512         # row block (matmul moving dim / one PSUM bank)
NR = R // RB     # row blocks per core (2)

# power-of-2 quantization scales
WS = 4096.0      # weight scale (2^12); max |w| ~0.031 -> 127 < 240
S_S = 256.0      # summary act scale (2^8); max ~0.16 -> 41
S_M1 = 64.0      # m1 act scale; max ~0.82 -> 52
S_M2 = 128.0     # m2 act scale; max ~0.36 -> 46
S_H1 = 64.0      # h1 act scale; max ~0.82 -> 52
PS_L1 = S_S * WS          # scale of layer-1 PSUM (2^20)

_BUILT = None


def _build():
    nc = bacc.Bacc("TRN2", target_bir_lowering=False, debug=False,
                   num_devices=N_CORES)
    f32 = mybir.dt.float32
    bf = mybir.dt.bfloat16
    f8 = mybir.dt.float8e4
    DR = mybir.MatmulPerfMode.DoubleRow

    towd = nc.declare_dram_parameter("tow", [K, NI, R], bf, isOutput=False)
    aggd = nc.declare_dram_parameter("agg", [128, FT], f8, isOutput=False)
    mw1sd = nc.declare_dram_parameter("mw1s", [NP, 128, 2, NH], f8, isOutput=False)
    mw1td = nc.declare_dram_parameter("mw1t", [NI, NH], bf, isOutput=False)
    mw2d = nc.declare_dram_parameter("mw2", [NP, 128, 2, NH], f8, isOutput=False)
    mw3d = nc.declare_dram_parameter("mw3", [NP, 128, 2, NH], f8, isOutput=False)
    ow1sd = nc.declare_dram_parameter("ow1s", [NP, 128, 2, NH], f8, isOutput=False)
    ow1td = nc.declare_dram_parameter("ow1t", [NI, NH], bf, isOutput=False)
    ow2d = nc.declare_dram_parameter("ow2", [NP, 128, 2, NH], f8, isOutput=False)
    w3cd = nc.declare_dram_parameter("w3c", [128, FT], f32, isOutput=False)
    balld = nc.declare_dram_parameter("ball", [128, 40], f32, isOutput=False)
    ob3d = nc.declare_dram_parameter("ob3", [1, 1], f32, isOutput=False)
    outd = nc.declare_dram_parameter("out", [1, R], f32, isOutput=True)

    Relu = mybir.ActivationFunctionType.Relu
    Sigmoid = mybir.ActivationFunctionType.Sigmoid
    Identity = mybir.ActivationFunctionType.Identity
    Add = mybir.AluOpType.add
    Mult = mybir.AluOpType.mult

    # epilogue scale constants: out_next = relu(psum * k + b * s_next)
    K_M1 = S_M1 / PS_L1
    K_O1 = S_H1 / PS_L1
    K_M2 = S_M2 / (S_M1 * WS)
    K_M3 = S_S / (S_M2 * WS)
    K_O2 = 1.0 / (S_H1 * WS)   # h2 stored in true units (bf16)

    with tile.TileContext(nc) as tc:
        with (
            tc.tile_pool(name="weights", bufs=1) as wp,
            tc.tile_pool(name="summary", bufs=1) as sp,
            tc.tile_pool(name="acts", bufs=8) as ap,
            tc.tile_pool(name="tow", bufs=4) as twp,
            tc.tile_pool(name="small", bufs=1) as smp,
            tc.tile_pool(name="zwork", bufs=2) as zw,
            tc.tile_pool(name="psum", bufs=6, space="PSUM") as pp,
            tc.tile_pool(name="zpsum", bufs=2, space="PSUM") as zp,
        ):
            # --- weights: fp8 contraction-pair tiles, spread across the
            # sync/vector DGE queues by first use (ACT carries all epilogues
            # so its sequencer stays clear of DMA issue).
            def load_w_split(dram, name, engs):
                tiles = []
                for i in range(NP):
                    t = wp.tile([128, 2, NH], f8, tag=f"{name}{i}",
                                name=f"{name}{i}")
                    engs[i % len(engs)].dma_start(out=t, in_=dram[i])
                    tiles.append(t)
                return tiles

            ball = smp.tile([128, 40], f32, tag="ball", name="ball")
            nc.gpsimd.dma_start(out=ball, in_=balld[:])
            ob3 = smp.tile([1, 1], f32, tag="ob3", name="ob3")
            nc.gpsimd.dma_start(out=ob3, in_=ob3d[:])
            aggt = smp.tile([128, FT], f8, tag="aggt", name="aggt")
            nc.gpsimd.dma_start(out=aggt, in_=aggd[:])
            w3c = smp.tile([128, FT], f32, tag="w3c", name="w3c")
            nc.gpsimd.dma_start(out=w3c, in_=w3cd[:])
            mw1t = wp.tile([NI, NH], bf, tag="mw1t", name="mw1t")
            nc.gpsimd.dma_start(out=mw1t, in_=mw1td[:])
            ow1t = wp.tile([128, NH], bf, tag="ow1t", name="ow1t")
            nc.gpsimd.memset(ow1t[64:128, :], 0.0)
            mw1s = load_w_split(mw1sd, "mw1s",
                                [nc.sync, nc.scalar, nc.gpsimd])
            ow1s = load_w_split(ow1sd, "ow1s",
                                [nc.sync, nc.scalar, nc.gpsimd])
            nc.gpsimd.dma_start(out=ow1t[64:128, :], in_=ow1td[:])
            mw2 = load_w_split(mw2d, "mw2", [nc.sync, nc.scalar])
            mw3 = load_w_split(mw3d, "mw3", [nc.sync, nc.scalar])
            ow2 = load_w_split(ow2d, "ow2", [nc.sync, nc.scalar])

            ones = smp.tile([128, 1], bf, tag="ones", name="ones")
            nc.vector.memset(ones, 1.0)
            onesrow = smp.tile([1, RB], bf, tag="onesrow", name="onesrow")
            nc.vector.memset(onesrow, 1.0)

            # --- summary double buffer: fp8 contraction pairs.  sA is never
            # read at k=0 (step-0 summary contribution is rank-1), so no
            # initialization is needed. ---
            sA = [[sp.tile([128, 2, RB], f8, tag=f"sA{i}_{r}",
                           name=f"sA{i}_{r}") for r in range(NR)]
                  for i in range(NP)]
            sB = [[sp.tile([128, 2, RB], f8, tag=f"sB{i}_{r}",
                           name=f"sB{i}_{r}") for r in range(NR)]
                  for i in range(NP)]

            # --- product accumulators ---
            pacc = []
            for r in range(NR):
                t = smp.tile([1, RB], f32, tag=f"pacc{r}", name=f"pacc{r}")
                nc.vector.memset(t, 1.0)
                pacc.append(t)

            # bias column index per layer: 0=Mb1 1=Mb2 2=Mb3 3=Ob1 4=Ob2
            def epilogue(ot, ps, bias_l, m, k):
                nc.scalar.activation(ot, ps[:], Relu,
                                     bias=ball[:, ds(bias_l * 8 + m, 1)],
                                     scale=k)

            def layer1(scur, tow_t, branches=("m", "o")):
                """Fused M/O layer 1.  Per (branch, m): 4 DoubleRow fp8
                matmuls over the summary pairs, closed by a bf16 tower
                matmul (M on PE rows 0-63, O on rows 64-127 so each M/O
                pair streams concurrently).  Two m-columns batched per pass
                so the partial-row LDWEIGHTS exposure amortizes."""
                m1o = [[None] * FT for _ in range(NR)]
                h1o = [[None] * FT for _ in range(NR)]
                for r in range(NR):
                    for mp in range(0, FT, 2):
                        psms, psos = [], []
                        for m in (mp, mp + 1):
                            if "m" in branches:
                                psm = pp.tile([128, RB], f32,
                                              tag="ps", name="psm")
                                psms.append(psm)
                                for i in range(NP):
                                    nc.tensor.matmul(
                                        psm[:], mw1s[i][:, :, ts(m, 128)],
                                        scur[i][r][:, :, :],
                                        start=(i == 0), stop=False,
                                        perf_mode=DR)
                            if "o" in branches:
                                pso = pp.tile([128, RB], f32,
                                              tag="ps", name="pso")
                                psos.append(pso)
                                for i in range(NP):
                                    nc.tensor.matmul(
                                        pso[:], ow1s[i][:, :, ts(m, 128)],
                                        scur[i][r][:, :, :],
                                        start=(i == 0), stop=False,
                                        perf_mode=DR)
                        for j, m in enumerate((mp, mp + 1)):
                            if "m" in branches:
                                nc.tensor.matmul(
                                    psms[j][:], mw1t[:, ts(m, 128)],
                                    tow_t[0:NI, ts(r, RB)],
                                    start=False, stop=True)
                            if "o" in branches:
                                nc.tensor.matmul(
                                    psos[j][:], ow1t[64:128, ts(m, 128)],
                                    tow_t[64:128, ts(r, RB)],
                                    start=False, stop=True)
                        for j, m in enumerate((mp, mp + 1)):
                            if "m" in branches:
                                if m % 2 == 0:
                                    mt = ap.tile([128, 2, RB], f8, tag="m1",
                                                 name="m1")
                                epilogue(mt[:, m % 2, :], psms[j], 0, m, K_M1)
                                m1o[r][m] = mt
                            if "o" in branches:
                                if m % 2 == 0:
                                    ht = ap.tile([128, 2, RB], f8, tag="h1",
                                                 name="h1")
                                epilogue(ht[:, m % 2, :], psos[j], 3, m, K_O1)
                                h1o[r][m] = ht
                # repack: pair tile list indexed [pair][r]
                pairs = lambda o: [[o[r][2 * p] for r in range(NR)]
                                   for p in range(NP)]
                return pairs(m1o), pairs(h1o)

            def layer(rhs, ws, bias_l, k, out_mode, out_tiles=None):
                """rhs: [NP][NR] fp8 pair tiles.  out_mode: 'pair' -> new fp8
                pair tiles, 'spair' -> write into out_tiles (summary pairs),
                'flat' -> bf16 flat tiles (h2)."""
                outs = [[None] * FT for _ in range(NR)]
                for r in range(NR):
                    cur = None
                    for m in range(FT):
                        ps = pp.tile([128, RB], f32, tag="ps", name="ps")
                        for i in range(NP):
                            nc.tensor.matmul(
                                ps[:], ws[i][:, :, ts(m, 128)],
                                rhs[i][r][:, :, :],
                                start=(i == 0), stop=(i == NP - 1),
                                perf_mode=DR)
                        if out_mode == "flat":
                            ot = ap.tile([128, RB], bf, tag="h2", name="h2",
                                         bufs=16)
                            epilogue(ot[:], ps, bias_l, m, k)
                            outs[r][m] = ot
                        else:
                            if out_mode == "spair":
                                cur = out_tiles[m // 2][r]
                            elif m % 2 == 0:
                                cur = ap.tile([128, 2, RB], f8, tag="l2",
                                              name="l2")
                            epilogue(cur[:, m % 2, :], ps, bias_l, m, k)
                            outs[r][m] = cur
                if out_mode == "flat":
                    return [[outs[r][m] for r in range(NR)]
                            for m in range(FT)]
                return [[outs[r][2 * p] for r in range(NR)]
                        for p in range(NP)]

            # Step-0 rank-1 trick: summary0 = broadcast(aggregate) is the
            # same for every row, so its layer-1 contribution is a single
            # vector v = agg @ W1s per branch; stored scaled by PS_L1 so the
            # k=0 PSUM groups carry the same scale as the generic path.
            # agg arrives fp8 scaled by 16 so the matmuls are fp8 x fp8.
            def compute_v(W, name):
                v_sb = smp.tile([1, NH], bf, tag=name, name=name)
                for half in range(2):
                    vp = pp.tile([1, RB], f32, tag="ps", name="vps")
                    for i in range(NP):
                        for jj in range(2):
                            nc.tensor.matmul(
                                vp[:], aggt[:, ds(2 * i + jj, 1)],
                                W[i][:, jj, ts(half, RB)],
                                start=(i == 0 and jj == 0),
                                stop=(i == NP - 1 and jj == 1))
                    # psum = 16 * WS * v ; store v * PS_L1 => scale S_S/16
                    nc.scalar.activation(
                        v_sb[0:1, ts(half, RB)], vp[:], Identity,
                        scale=S_S / 16.0)
                return v_sb

            def layer1_k0(tow_t, v_m, v_o):
                m1o = [[None] * FT for _ in range(NR)]
                h1o = [[None] * FT for _ in range(NR)]
                for br in ("m", "o"):
                    v_sb = v_m if br == "m" else v_o
                    for r in range(NR):
                        cur = None
                        for m in range(FT):
                            ps = pp.tile([128, RB], f32, tag="ps",
                                         name="psk0")
                            nc.tensor.matmul(
                                ps[:], v_sb[0:1, ts(m, 128)], onesrow[:],
                                start=True, stop=False)
                            if br == "m":
                                nc.tensor.matmul(
                                    ps[:], mw1t[:, ts(m, 128)],
                                    tow_t[0:NI, ts(r, RB)],
                                    start=False, stop=True)
                                if m % 2 == 0:
                                    cur = ap.tile([128, 2, RB], f8,
                                                  tag="m1", name="m1")
                                epilogue(cur[:, m % 2, :], ps, 0, m, K_M1)
                                m1o[r][m] = cur
                            else:
                                nc.tensor.matmul(
                                    ps[:], ow1t[64:128, ts(m, 128)],
                                    tow_t[64:128, ts(r, RB)],
                                    start=False, stop=True)
                                if m % 2 == 0:
                                    cur = ap.tile([128, 2, RB], f8,
                                                  tag="h1", name="h1")
                                epilogue(cur[:, m % 2, :], ps, 3, m, K_O1)
                                h1o[r][m] = cur
                pairs = lambda o: [[o[r][2 * p] for r in range(NR)]
                                   for p in range(NP)]
                return pairs(m1o), pairs(h1o)

            def flush_zjobs(zjobs):
                for gb, r in zjobs:
                    zps = zp.tile([1, RB], f32, tag="z", name="zps")
                    nc.tensor.matmul(zps[:], ones[:], gb[:],
                                     start=True, stop=True)
                    pr = smp.tile([1, RB], f32, tag=f"pr{r}",
                                  name=f"pr{r}")
                    nc.scalar.activation(pr[:], zps[:], Sigmoid, bias=ob3[:])
                    nc.vector.tensor_mul(pacc[r][:], pacc[r][:], pr[:])

            scur, snxt = sA, sB
            zjobs = []
            for k in range(K):
                tow_t = twp.tile([128, R], bf, tag="tow", name="tow")
                nc.gpsimd.dma_start(out=tow_t[0:NI, :], in_=towd[k])
                nc.gpsimd.dma_start(out=tow_t[64:128, :], in_=towd[k])

                if k == 0:
                    v_m = compute_v(mw1s, "v_m")
                    v_o = compute_v(ow1s, "v_o")
                    m1, h1 = layer1_k0(tow_t, v_m, v_o)
                elif k == K - 1:
                    # the final scan carry is discarded by the reference, so
                    # the last step's M branch (M1/M2/M3) is dead code
                    m1, h1 = layer1(scur, tow_t, branches=("o",))
                else:
                    m1, h1 = layer1(scur, tow_t)
                if k < K - 1:
                    m2 = layer(m1, mw2, 1, K_M2, "pair")
                    # previous step's output head (its DVE reduce is long
                    # done, so the sigmoid never head-of-line-blocks ACT)
                    flush_zjobs(zjobs)
                    zjobs = []
                    layer(m2, mw3, 2, K_M3, "spair", out_tiles=snxt)
                else:
                    flush_zjobs(zjobs)
                    zjobs = []
                h2 = layer(h1, ow2, 4, K_O2, "flat")
                # g = sum_i h2_i * w3_i on the DVE (per-partition scalars),
                # reduced across partitions next step by a ones-matmul.
                for r in range(NR):
                    if k < K - 1:
                        # DVE-serial chain; latency is hidden by the next
                        # step's PE work
                        g = zw.tile([128, RB], f32, tag="g", name="g")
                        nc.vector.tensor_scalar(
                            g[:], h2[0][r][:], w3c[:, ds(0, 1)], None, Mult)
                        for i in range(1, FT):
                            t = zw.tile([128, RB], f32,
                                        tag="t", name="t", bufs=3)
                            nc.vector.tensor_scalar(
                                t[:], h2[i][r][:], w3c[:, ds(i, 1)], None,
                                Mult)
                            nc.vector.tensor_tensor(g[:], g[:], t[:], Add)
                    else:
                        # final step: the chain is on the kernel's critical
                        # tail, so pipeline the multiplies on the scalar
                        # engine against the DVE adds
                        tts = []
                        g = None
                        for i in range(FT):
                            t = zw.tile([128, RB], f32,
                                        tag="t", name="t", bufs=3)
                            nc.scalar.activation(t[:], h2[i][r][:],
                                                 Identity,
                                                 scale=w3c[:, ds(i, 1)])
                            tts.append(t)
                            if i == 1:
                                g = zw.tile([128, RB], f32, tag="g",
                                            name="g")
                                nc.vector.tensor_tensor(
                                    g[:], tts[0][:], tts[1][:], Add)
                            elif i >= 2:
                                nc.vector.tensor_tensor(g[:], g[:], t[:],
                                                        Add)
                    gb = zw.tile([128, RB], bf, tag="gb", name="gb", bufs=4)
                    nc.vector.tensor_copy(gb[:], g[:])
                    zjobs.append((gb, r))

                scur, snxt = snxt, scur
            flush_zjobs(zjobs)

            for r in range(NR):
                nc.sync.dma_start(out=outd[:, ts(r, RB)], in_=pacc[r][:])

    nc.finalize()
    return nc


def _get_nc():
    global _BUILT
    if _BUILT is None:
        _BUILT = _build()
    return _BUILT


def _pack_pairs(W, scale):
    """[1024, NH] f32 -> [NP, 128, 2, NH] fp8 contraction pairs."""
    Wq = (np.asarray(W, np.float32) * scale).astype(FP8)
    return np.ascontiguousarray(
        Wq.reshape(NP, 2, 128, NH).transpose(0, 2, 1, 3))


def _prep_inputs(inputs):
    f32 = np.float32
    towers = np.asarray(inputs["towers"], dtype=f32)
    agg = np.asarray(inputs["aggregate"], dtype=f32)
    MW1 = np.asarray(inputs["MW1"], dtype=f32)
    OW1 = np.asarray(inputs["OW1"], dtype=f32)

    biases = []
    for bname, s in (("Mb1", S_M1), ("Mb2", S_M2), ("Mb3", S_S),
                     ("Ob1", S_H1), ("Ob2", 1.0)):
        biases.append((np.asarray(inputs[bname], f32) * s).reshape(FT, 128).T)

    shared = {
        "agg": np.ascontiguousarray(agg.reshape(FT, 128).T * 16.0).astype(FP8),
        "mw1s": _pack_pairs(MW1[:NH], WS),
        "mw1t": np.ascontiguousarray(MW1[NH:] * PS_L1).astype(BF16),
        "mw2": _pack_pairs(inputs["MW2"], WS),
        "mw3": _pack_pairs(inputs["MW3"], WS),
        "ow1s": _pack_pairs(OW1[:NH], WS),
        "ow1t": np.ascontiguousarray(OW1[NH:] * PS_L1).astype(BF16),
        "ow2": _pack_pairs(inputs["OW2"], WS),
        "w3c": np.ascontiguousarray(
            np.asarray(inputs["OW3"], f32).reshape(FT, 128).T),
        "ball": np.ascontiguousarray(np.concatenate(biases, axis=1)),
        "ob3": np.asarray(inputs["Ob3"], f32).reshape(1, 1),
    }
    in_maps = []
    for c in range(N_CORES):
        tc_ = towers[c * R:(c + 1) * R]          # (R, K, NI)
        towT = np.ascontiguousarray(tc_.transpose(1, 2, 0)).astype(BF16)
        in_maps.append({"tow": towT, **shared})
    return in_maps


def _run(inputs, trace=False):
    nc = _get_nc()
    in_maps = _prep_inputs(inputs)
    res = run_bass_kernel_spmd(nc, in_maps, list(range(N_CORES)), trace=trace)
    out = np.concatenate([res.results[c]["out"][0] for c in range(N_CORES)])
    return out.astype(np.float32), res


def kernel(**inputs):
    out, _ = _run(inputs, trace=False)
    return out


# revision 17
# speedup vs baseline: 1.0112x; 1.0112x over previous
"""Trainium2 Bass kernel for nn_BottomUpNet (dense_mlp).

Reference computation (per row n of N=8192, fully independent across rows):
    summary = aggregate (broadcast)                   # (1024,)
    for k in 0..15:
        x = [summary, towers[n, k, :]]                # (1088,)
        h = relu(x @ OW1 + Ob1); h = relu(h @ OW2 + Ob2)
        pred_k = sigmoid(h @ OW3 + Ob3)
        m = relu(x @ MW1 + Mb1); m = relu(m @ MW2 + Mb2); m = relu(m @ MW3 + Mb3)
        summary = m
    out[n] = prod_k pred_k

Strategy: data-parallel over N across 8 cores (1024 rows each), weights
replicated.  Activations are feature-major ([feature partition, row free]).

fp8 DoubleRow: all five big matmuls (M1s/O1s summary parts, M2, M3, O2) run
in fp8e4 (e4m3) with MatmulPerfMode.DoubleRow -- the PE processes two
128-deep contraction blocks per pass, 2x the bf16 MAC rate.  Weights are
packed host-side into contraction pairs [4][128, 2, 1024] scaled by 2^12;
activations are stored as fp8 pair tiles [128, 2, 512] with per-tensor
power-of-2 scales (summary 2^8, m1 2^6, m2 2^7, h1 2^6).  The 64-deep tower
matmuls stay bf16 (2.4% of cycles) with their weights pre-scaled by
s_act*2^12 so each PSUM group accumulates in one consistent scale; the
single scalar-engine epilogue then applies relu(psum*k + bias*s_next) and
writes fp8 directly.  Expected accuracy ~7e-3 max rel err (numpy sim), well
under the 2e-2 gate; expected time ~1.15ms vs 2.25ms bf16 baseline.

Other structure (inherited from the bf16 baseline):
  - step 0's summary contribution is rank-1 (broadcast aggregate): computed
    once as v = agg @ W1s and injected per tile by a contraction-1 matmul
    against a ones row.
  - layer-1 tower matmuls for the M/O branches pair into disjoint PE row
    groups (0-63 / 64-127) so they stream concurrently.
  - the 1024->1 output head is a DVE per-partition multiply/add tree over
    bf16 h2 plus a ones-vector matmul for the cross-partition reduce; its
    sigmoid + product-accumulate are deferred into the next step.
  - the final step's M branch is dead (scan carry discarded) and skipped.
"""

import numpy as np
import ml_dtypes

import concourse.bacc as bacc
import concourse.mybir as mybir
import concourse.tile as tile
from concourse.bass import ts, ds
from concourse.bass_utils import run_bass_kernel_spmd

BF16 = ml_dtypes.bfloat16
FP8 = ml_dtypes.float8_e4m3

N_CORES = 8
N = 8192
K = 16
NI = 64          # tower features per step
NH = 1024        # hidden width
FT = NH // 128   # feature tiles (8)
NP = FT // 2     # contraction pairs (4)
R = N // N_CORES  # rows per core (1024)
RB = 512         # row block (matmul moving dim / one PSUM bank)
NR = R // RB     # row blocks per core (2)

# power-of-2 quantization scales
WS = 4096.0      # weight scale (2^12); max |w| ~0.031 -> 127 < 240
S_S = 256.0      # summary act scale (2^8); max ~0.16 -> 41
S_M1 = 64.0      # m1 act scale; max ~0.82 -> 52
S_M2 = 128.0     # m2 act scale; max ~0.36 -> 46
S_H1 = 64.0      # h1 act scale; max ~0.82 -> 52
PS_L1 = S_S * WS          # scale of layer-1 PSUM (2^20)

_BUILT = None


def _build():
    nc = bacc.Bacc("TRN2", target_bir_lowering=False, debug=False,
                   num_devices=N_CORES)
    f32 = mybir.dt.float32
    bf = mybir.dt.bfloat16
    f8 = mybir.dt.float8e4
    DR = mybir.MatmulPerfMode.DoubleRow

    towd = nc.declare_dram_parameter("tow", [K, NI, R], bf, isOutput=False)
    vmd = nc.declare_dram_parameter("v_m", [1, NH], bf, isOutput=False)
    vod = nc.declare_dram_parameter("v_o", [1, NH], bf, isOutput=False)
    mw1sd = nc.declare_dram_parameter("mw1s", [NP, 128, 2, NH], f8, isOutput=False)
    mw1td = nc.declare_dram_parameter("mw1t", [NI, NH], bf, isOutput=False)
    mw2d = nc.declare_dram_parameter("mw2", [NP, 128, 2, NH], f8, isOutput=False)
    mw3d = nc.declare_dram_parameter("mw3", [NP, 128, 2, NH], f8, isOutput=False)
    ow1sd = nc.declare_dram_parameter("ow1s", [NP, 128, 2, NH], f8, isOutput=False)
    ow1td = nc.declare_dram_parameter("ow1t", [NI, NH], bf, isOutput=False)
    ow2d = nc.declare_dram_parameter("ow2", [NP, 128, 2, NH], f8, isOutput=False)
    w3cd = nc.declare_dram_parameter("w3c", [128, FT], f32, isOutput=False)
    w3bd = nc.declare_dram_parameter("w3b", [128, FT], bf, isOutput=False)
    balld = nc.declare_dram_parameter("ball", [128, 40], f32, isOutput=False)
    ob3d = nc.declare_dram_parameter("ob3", [1, 1], f32, isOutput=False)
    outd = nc.declare_dram_parameter("out", [1, R], f32, isOutput=True)

    Relu = mybir.ActivationFunctionType.Relu
    Sigmoid = mybir.ActivationFunctionType.Sigmoid
    Identity = mybir.ActivationFunctionType.Identity
    Add = mybir.AluOpType.add
    Mult = mybir.AluOpType.mult

    # epilogue scale constants: out_next = relu(psum * k + b * s_next)
    K_M1 = S_M1 / PS_L1
    K_O1 = S_H1 / PS_L1
    K_M2 = S_M2 / (S_M1 * WS)
    K_M3 = S_S / (S_M2 * WS)
    K_O2 = 1.0 / (S_H1 * WS)   # h2 stored in true units (bf16)

    with tile.TileContext(nc) as tc:
        with (
            tc.tile_pool(name="weights", bufs=1) as wp,
            tc.tile_pool(name="summary", bufs=1) as sp,
            tc.tile_pool(name="acts", bufs=8) as ap,
            tc.tile_pool(name="tow", bufs=4) as twp,
            tc.tile_pool(name="small", bufs=1) as smp,
            tc.tile_pool(name="zwork", bufs=2) as zw,
            tc.tile_pool(name="psum", bufs=6, space="PSUM") as pp,
            tc.tile_pool(name="zpsum", bufs=2, space="PSUM") as zp,
        ):
            # --- weights: fp8 contraction-pair tiles, spread across the
            # sync/vector DGE queues by first use (ACT carries all epilogues
            # so its sequencer stays clear of DMA issue).
            def load_w_split(dram, name, engs):
                tiles = []
                for i in range(NP):
                    t = wp.tile([128, 2, NH], f8, tag=f"{name}{i}",
                                name=f"{name}{i}")
                    engs[i % len(engs)].dma_start(out=t, in_=dram[i])
                    tiles.append(t)
                return tiles

            # k=0 needs only the smalls + tower weights + v vectors: the big
            # fp8 summary weights (mw1s/ow1s) are first read at k=1, so they
            # load dead last and never gate the PE start.
            ball = smp.tile([128, 40], f32, tag="ball", name="ball")
            nc.gpsimd.dma_start(out=ball, in_=balld[:])
            ob3 = smp.tile([1, 1], f32, tag="ob3", name="ob3")
            nc.gpsimd.dma_start(out=ob3, in_=ob3d[:])
            v_m = smp.tile([1, NH], bf, tag="v_m", name="v_m")
            nc.gpsimd.dma_start(out=v_m, in_=vmd[:])
            v_o = smp.tile([1, NH], bf, tag="v_o", name="v_o")
            nc.gpsimd.dma_start(out=v_o, in_=vod[:])
            w3c = smp.tile([128, FT], f32, tag="w3c", name="w3c")
            nc.gpsimd.dma_start(out=w3c, in_=w3cd[:])
            w3b = smp.tile([128, FT], bf, tag="w3b", name="w3b")
            nc.gpsimd.dma_start(out=w3b, in_=w3bd[:])
            mw1t = wp.tile([NI, NH], bf, tag="mw1t", name="mw1t")
            nc.sync.dma_start(out=mw1t, in_=mw1td[:])
            ow1t = wp.tile([128, NH], bf, tag="ow1t", name="ow1t")
            nc.gpsimd.memset(ow1t[64:128, :], 0.0)
            nc.scalar.dma_start(out=ow1t[64:128, :], in_=ow1td[:])
            mw2 = load_w_split(mw2d, "mw2", [nc.sync, nc.scalar])
            mw3 = load_w_split(mw3d, "mw3", [nc.sync, nc.scalar])
            ow2 = load_w_split(ow2d, "ow2", [nc.sync, nc.scalar])
            mw1s = load_w_split(mw1sd, "mw1s",
                                [nc.sync, nc.scalar, nc.gpsimd])
            ow1s = load_w_split(ow1sd, "ow1s",
                                [nc.sync, nc.scalar, nc.gpsimd])

            ones = smp.tile([128, 1], bf, tag="ones", name="ones")
            nc.vector.memset(ones, 1.0)
            onesrow = smp.tile([1, RB], bf, tag="onesrow", name="onesrow")
            nc.vector.memset(onesrow, 1.0)

            # --- summary double buffer: fp8 contraction pairs.  sA is never
            # read at k=0 (step-0 summary contribution is rank-1), so no
            # initialization is needed. ---
            sA = [[sp.tile([128, 2, RB], f8, tag=f"sA{i}_{r}",
                           name=f"sA{i}_{r}") for r in range(NR)]
                  for i in range(NP)]
            sB = [[sp.tile([128, 2, RB], f8, tag=f"sB{i}_{r}",
                           name=f"sB{i}_{r}") for r in range(NR)]
                  for i in range(NP)]

            # --- product accumulators ---
            pacc = []
            for r in range(NR):
                t = smp.tile([1, RB], f32, tag=f"pacc{r}", name=f"pacc{r}")
                nc.vector.memset(t, 1.0)
                pacc.append(t)

            # bias column index per layer: 0=Mb1 1=Mb2 2=Mb3 3=Ob1 4=Ob2
            def epilogue(ot, ps, bias_l, m, k):
                nc.scalar.activation(ot, ps[:], Relu,
                                     bias=ball[:, ds(bias_l * 8 + m, 1)],
                                     scale=k)

            def layer1(scur, tow_t, branches=("m", "o")):
                """Fused M/O layer 1.  Per (branch, m): 4 DoubleRow fp8
                matmuls over the summary pairs, closed by a bf16 tower
                matmul (M on PE rows 0-63, O on rows 64-127 so each M/O
                pair streams concurrently).  Two m-columns batched per pass
                so the partial-row LDWEIGHTS exposure amortizes."""
                m1o = [[None] * FT for _ in range(NR)]
                h1o = [[None] * FT for _ in range(NR)]
                for r in range(NR):
                    for mp in range(0, FT, 2):
                        psms, psos = [], []
                        for m in (mp, mp + 1):
                            if "m" in branches:
                                psm = pp.tile([128, RB], f32,
                                              tag="ps", name="psm")
                                psms.append(psm)
                                for i in range(NP):
                                    nc.tensor.matmul(
                                        psm[:], mw1s[i][:, :, ts(m, 128)],
                                        scur[i][r][:, :, :],
                                        start=(i == 0), stop=False,
                                        perf_mode=DR)
                            if "o" in branches:
                                pso = pp.tile([128, RB], f32,
                                              tag="ps", name="pso")
                                psos.append(pso)
                                for i in range(NP):
                                    nc.tensor.matmul(
                                        pso[:], ow1s[i][:, :, ts(m, 128)],
                                        scur[i][r][:, :, :],
                                        start=(i == 0), stop=False,
                                        perf_mode=DR)
                        for j, m in enumerate((mp, mp + 1)):
                            if "m" in branches:
                                nc.tensor.matmul(
                                    psms[j][:], mw1t[:, ts(m, 128)],
                                    tow_t[0:NI, ts(r, RB)],
                                    start=False, stop=True)
                            if "o" in branches:
                                nc.tensor.matmul(
                                    psos[j][:], ow1t[64:128, ts(m, 128)],
                                    tow_t[64:128, ts(r, RB)],
                                    start=False, stop=True)
                        for j, m in enumerate((mp, mp + 1)):
                            if "m" in branches:
                                if m % 2 == 0:
                                    mt = ap.tile([128, 2, RB], f8, tag="m1",
                                                 name="m1")
                                epilogue(mt[:, m % 2, :], psms[j], 0, m, K_M1)
                                m1o[r][m] = mt
                            if "o" in branches:
                                if m % 2 == 0:
                                    ht = ap.tile([128, 2, RB], f8, tag="h1",
                                                 name="h1")
                                epilogue(ht[:, m % 2, :], psos[j], 3, m, K_O1)
                                h1o[r][m] = ht
                # repack: pair tile list indexed [pair][r]
                pairs = lambda o: [[o[r][2 * p] for r in range(NR)]
                                   for p in range(NP)]
                return pairs(m1o), pairs(h1o)

            def layer(rhs, ws, bias_l, k, out_mode, out_tiles=None):
                """rhs: [NP][NR] fp8 pair tiles.  out_mode: 'pair' -> new fp8
                pair tiles, 'spair' -> write into out_tiles (summary pairs),
                'flat' -> bf16 flat tiles (h2)."""
                outs = [[None] * FT for _ in range(NR)]
                for r in range(NR):
                    cur = None
                    for m in range(FT):
                        ps = pp.tile([128, RB], f32, tag="ps", name="ps")
                        for i in range(NP):
                            nc.tensor.matmul(
                                ps[:], ws[i][:, :, ts(m, 128)],
                                rhs[i][r][:, :, :],
                                start=(i == 0), stop=(i == NP - 1),
                                perf_mode=DR)
                        if out_mode == "flat":
                            ot = ap.tile([128, RB], bf, tag="h2", name="h2",
                                         bufs=16)
                            epilogue(ot[:], ps, bias_l, m, k)
                            outs[r][m] = ot
                        else:
                            if out_mode == "spair":
                                cur = out_tiles[m // 2][r]
                            elif m % 2 == 0:
                                cur = ap.tile([128, 2, RB], f8, tag="l2",
                                              name="l2")
                            epilogue(cur[:, m % 2, :], ps, bias_l, m, k)
                            outs[r][m] = cur
                if out_mode == "flat":
                    return [[outs[r][m] for r in range(NR)]
                            for m in range(FT)]
                return [[outs[r][2 * p] for r in range(NR)]
                        for p in range(NP)]

            # Step-0 rank-1 trick: summary0 = broadcast(aggregate) is the
            # same for every row, so its layer-1 contribution is a single
            # vector v = agg @ W1s per branch -- precomputed exactly on the
            # host (it is a 1-row matvec over inputs only) and shipped
            # scaled by PS_L1 so the k=0 PSUM groups carry the same scale
            # as the generic path.
            def layer1_k0(tow_t, v_m, v_o):
                m1o = [[None] * FT for _ in range(NR)]
                h1o = [[None] * FT for _ in range(NR)]
                for br in ("m", "o"):
                    v_sb = v_m if br == "m" else v_o
                    for r in range(NR):
                        cur = None
                        for m in range(FT):
                            ps = pp.tile([128, RB], f32, tag="ps",
                                         name="psk0")
                            nc.tensor.matmul(
                                ps[:], v_sb[0:1, ts(m, 128)], onesrow[:],
                                start=True, stop=False)
                            if br == "m":
                                nc.tensor.matmul(
                                    ps[:], mw1t[:, ts(m, 128)],
                                    tow_t[0:NI, ts(r, RB)],
                                    start=False, stop=True)
                                if m % 2 == 0:
                                    cur = ap.tile([128, 2, RB], f8,
                                                  tag="m1", name="m1")
                                epilogue(cur[:, m % 2, :], ps, 0, m, K_M1)
                                m1o[r][m] = cur
                            else:
                                nc.tensor.matmul(
                                    ps[:], ow1t[64:128, ts(m, 128)],
                                    tow_t[64:128, ts(r, RB)],
                                    start=False, stop=True)
                                if m % 2 == 0:
                                    cur = ap.tile([128, 2, RB], f8,
                                                  tag="h1", name="h1")
                                epilogue(cur[:, m % 2, :], ps, 3, m, K_O1)
                                h1o[r][m] = cur
                pairs = lambda o: [[o[r][2 * p] for r in range(NR)]
                                   for p in range(NP)]
                return pairs(m1o), pairs(h1o)

            def flush_zjobs(zjobs):
                for gb, r in zjobs:
                    zps = zp.tile([1, RB], f32, tag="z", name="zps")
                    nc.tensor.matmul(zps[:], ones[:], gb[:],
                                     start=True, stop=True)
                    pr = smp.tile([1, RB], f32, tag=f"pr{r}",
                                  name=f"pr{r}")
                    nc.scalar.activation(pr[:], zps[:], Sigmoid, bias=ob3[:])
                    nc.vector.tensor_mul(pacc[r][:], pacc[r][:], pr[:])

            scur, snxt = sA, sB
            zjobs = []
            for k in range(K):
                tow_t = twp.tile([128, R], bf, tag="tow", name="tow")
                nc.gpsimd.dma_start(out=tow_t[0:NI, :], in_=towd[k])
                nc.gpsimd.dma_start(out=tow_t[64:128, :], in_=towd[k])

                if k == 0:
                    m1, h1 = layer1_k0(tow_t, v_m, v_o)
                elif k == K - 1:
                    # the final scan carry is discarded by the reference, so
                    # the last step's M branch (M1/M2/M3) is dead code
                    m1, h1 = layer1(scur, tow_t, branches=("o",))
                else:
                    m1, h1 = layer1(scur, tow_t)
                if k < K - 1:
                    m2 = layer(m1, mw2, 1, K_M2, "pair")
                    # previous step's output head (its DVE reduce is long
                    # done, so the sigmoid never head-of-line-blocks ACT)
                    flush_zjobs(zjobs)
                    zjobs = []
                    layer(m2, mw3, 2, K_M3, "spair", out_tiles=snxt)
                else:
                    flush_zjobs(zjobs)
                    zjobs = []
                h2 = layer(h1, ow2, 4, K_O2, "flat")
                # g = sum_i h2_i * w3_i on the DVE (per-partition scalars),
                # reduced across partitions next step by a ones-matmul.
                for r in range(NR):
                    if k < K - 1:
                        # DVE-serial chain; latency is hidden by the next
                        # step's PE work
                        g = zw.tile([128, RB], f32, tag="g", name="g")
                        nc.vector.tensor_scalar(
                            g[:], h2[0][r][:], w3c[:, ds(0, 1)], None, Mult)
                        for i in range(1, FT):
                            t = zw.tile([128, RB], f32,
                                        tag="t", name="t", bufs=3)
                            nc.vector.tensor_scalar(
                                t[:], h2[i][r][:], w3c[:, ds(i, 1)], None,
                                Mult)
                            nc.vector.tensor_tensor(g[:], g[:], t[:], Add)
                        gb = zw.tile([128, RB], bf, tag="gb", name="gb",
                                     bufs=4)
                        nc.vector.tensor_copy(gb[:], g[:])
                        zjobs.append((gb, r))
                    else:
                        # final step: the PE is idle by now, so the whole
                        # 1024->1 reduce runs as 8 accumulating matmuls
                        # (bf16 w3 columns as stationary) straight into a
                        # [1, RB] PSUM, skipping the DVE chain latency on
                        # the kernel's critical tail.
                        zps = zp.tile([1, RB], f32, tag="z", name="zps")
                        for i in range(FT):
                            nc.tensor.matmul(
                                zps[:], w3b[:, ds(i, 1)], h2[i][r][:],
                                start=(i == 0), stop=(i == FT - 1))
                        pr = smp.tile([1, RB], f32, tag=f"pr{r}",
                                      name=f"pr{r}")
                        nc.scalar.activation(pr[:], zps[:], Sigmoid,
                                             bias=ob3[:])
                        nc.vector.tensor_mul(pacc[r][:], pacc[r][:],
                                             pr[:])

                scur, snxt = snxt, scur
            flush_zjobs(zjobs)

            for r in range(NR):
                nc.sync.dma_start(out=outd[:, ts(r, RB)], in_=pacc[r][:])

    nc.finalize()
    return nc


def _get_nc():
    global _BUILT
    if _BUILT is None:
        _BUILT = _build()
    return _BUILT


def _pack_pairs(W, scale):
    """[1024, NH] f32 -> [NP, 128, 2, NH] fp8 contraction pairs."""
    Wq = (np.asarray(W, np.float32) * scale).astype(FP8)
    return np.ascontiguousarray(
        Wq.reshape(NP, 2, 128, NH).transpose(0, 2, 1, 3))


def _prep_inputs(inputs):
    f32 = np.float32
    towers = np.asarray(inputs["towers"], dtype=f32)
    agg = np.asarray(inputs["aggregate"], dtype=f32)
    MW1 = np.asarray(inputs["MW1"], dtype=f32)
    OW1 = np.asarray(inputs["OW1"], dtype=f32)

    biases = []
    for bname, s in (("Mb1", S_M1), ("Mb2", S_M2), ("Mb3", S_S),
                     ("Ob1", S_H1), ("Ob2", 1.0)):
        biases.append((np.asarray(inputs[bname], f32) * s).reshape(FT, 128).T)

    # step-0 rank-1 layer-1 contribution, exact on host (1-row matvec)
    v_m = (agg.reshape(1, NH) @ MW1[:NH]) * PS_L1
    v_o = (agg.reshape(1, NH) @ OW1[:NH]) * PS_L1

    shared = {
        "v_m": v_m.astype(BF16),
        "v_o": v_o.astype(BF16),
        "mw1s": _pack_pairs(MW1[:NH], WS),
        "mw1t": np.ascontiguousarray(MW1[NH:] * PS_L1).astype(BF16),
        "mw2": _pack_pairs(inputs["MW2"], WS),
        "mw3": _pack_pairs(inputs["MW3"], WS),
        "ow1s": _pack_pairs(OW1[:NH], WS),
        "ow1t": np.ascontiguousarray(OW1[NH:] * PS_L1).astype(BF16),
        "ow2": _pack_pairs(inputs["OW2"], WS),
        "w3c": np.ascontiguousarray(
            np.asarray(inputs["OW3"], f32).reshape(FT, 128).T),
        "w3b": np.ascontiguousarray(
            np.asarray(inputs["OW3"], f32).reshape(FT, 128).T).astype(BF16),
        "ball": np.ascontiguousarray(np.concatenate(biases, axis=1)),
        "ob3": np.asarray(inputs["Ob3"], f32).reshape(1, 1),
    }
    in_maps = []
    for c in range(N_CORES):
        tc_ = towers[c * R:(c + 1) * R]          # (R, K, NI)
        towT = np.ascontiguousarray(tc_.transpose(1, 2, 0)).astype(BF16)
        in_maps.append({"tow": towT, **shared})
    return in_maps


def _run(inputs, trace=False):
    nc = _get_nc()
    in_maps = _prep_inputs(inputs)
    res = run_bass_kernel_spmd(nc, in_maps, list(range(N_CORES)), trace=trace)
    out = np.concatenate([res.results[c]["out"][0] for c in range(N_CORES)])
    return out.astype(np.float32), res


def kernel(**inputs):
    out, _ = _run(inputs, trace=False)
    return out


# revision 19
# speedup vs baseline: 1.0157x; 1.0044x over previous
"""Trainium2 Bass kernel for nn_BottomUpNet (dense_mlp).

Reference computation (per row n of N=8192, fully independent across rows):
    summary = aggregate (broadcast)                   # (1024,)
    for k in 0..15:
        x = [summary, towers[n, k, :]]                # (1088,)
        h = relu(x @ OW1 + Ob1); h = relu(h @ OW2 + Ob2)
        pred_k = sigmoid(h @ OW3 + Ob3)
        m = relu(x @ MW1 + Mb1); m = relu(m @ MW2 + Mb2); m = relu(m @ MW3 + Mb3)
        summary = m
    out[n] = prod_k pred_k

Strategy: data-parallel over N across 8 cores (1024 rows each), weights
replicated.  Activations are feature-major ([feature partition, row free]).

fp8 DoubleRow: all five big matmuls (M1s/O1s summary parts, M2, M3, O2) run
in fp8e4 (e4m3) with MatmulPerfMode.DoubleRow -- the PE processes two
128-deep contraction blocks per pass, 2x the bf16 MAC rate.  Weights are
packed host-side into contraction pairs [4][128, 2, 1024] scaled by 2^12;
activations live in fp8 pair tiles [128, 2(pair), 2(rowblk), 512] with
per-tensor power-of-2 scales (summary 2^8, m1 2^6, m2 2^7, h1 2^6).  The
64-deep tower matmuls stay bf16 with weights pre-scaled by s_act*2^12 so
each PSUM group accumulates in one consistent scale.

PSUM tiles are double-bank [128, 2(rowblk), 512]: the two row-block groups
of each output tile fill adjacent banks (with the stationary weights reused
back-to-back), and ONE scalar-engine epilogue relu(psum*k + bias*s_next)
drains both, halving ACT occupancy so it never backs up the PE's PSUM
rotation.  Measured accuracy 7.2e-3 max rel err vs the 2e-2 gate.

Other structure:
  - step 0's summary contribution is rank-1 (broadcast aggregate): v =
    agg @ W1s is precomputed exactly on the host (1-row matvec over inputs
    only) and injected per tile by a contraction-1 matmul against a ones
    row; mw1s/ow1s then load dead last, never gating the PE start.
  - layer-1 tower matmuls for the M/O branches pair into disjoint PE row
    groups (0-63 / 64-127) so they stream concurrently.
  - the 1024->1 output head is a DVE per-partition multiply/add tree over
    bf16 h2 plus a ones-vector matmul for the cross-partition reduce; its
    sigmoid + product-accumulate are deferred into the next step.  The
    final step's head instead runs as 8 accumulating w3-column matmuls on
    the by-then-idle PE, cutting the tail latency.
  - the final step's M branch is dead (scan carry discarded) and skipped.
"""

import numpy as np
import ml_dtypes

import concourse.bacc as bacc
import concourse.mybir as mybir
import concourse.tile as tile
from concourse.bass import ts, ds
from concourse.bass_utils import run_bass_kernel_spmd

BF16 = ml_dtypes.bfloat16
FP8 = ml_dtypes.float8_e4m3

N_CORES = 8
N = 8192
K = 16
NI = 64          # tower features per step
NH = 1024        # hidden width
FT = NH // 128   # feature tiles (8)
NP = FT // 2     # contraction pairs (4)
R = N // N_CORES  # rows per core (1024)
RB = 512         # row block (matmul moving dim / one PSUM bank)
NR = R // RB     # row blocks per core (2)

# power-of-2 quantization scales
WS = 4096.0      # weight scale (2^12); max |w| ~0.031 -> 127 < 240
S_S = 256.0      # summary act scale (2^8); max ~0.16 -> 41
S_M1 = 64.0      # m1 act scale; max ~0.82 -> 52
S_M2 = 128.0     # m2 act scale; max ~0.36 -> 46
S_H1 = 64.0      # h1 act scale; max ~0.82 -> 52
PS_L1 = S_S * WS          # scale of layer-1 PSUM (2^20)

_BUILT = None


def _build():
    nc = bacc.Bacc("TRN2", target_bir_lowering=False, debug=False,
                   num_devices=N_CORES)
    f32 = mybir.dt.float32
    bf = mybir.dt.bfloat16
    f8 = mybir.dt.float8e4
    DR = mybir.MatmulPerfMode.DoubleRow

    towd = nc.declare_dram_parameter("tow", [K, NI, R], bf, isOutput=False)
    vmd = nc.declare_dram_parameter("v_m", [1, NH], bf, isOutput=False)
    vod = nc.declare_dram_parameter("v_o", [1, NH], bf, isOutput=False)
    mw1sd = nc.declare_dram_parameter("mw1s", [NP, 128, 2, NH], f8, isOutput=False)
    mw1td = nc.declare_dram_parameter("mw1t", [NI, NH], bf, isOutput=False)
    mw2d = nc.declare_dram_parameter("mw2", [NP, 128, 2, NH], f8, isOutput=False)
    mw3d = nc.declare_dram_parameter("mw3", [NP, 128, 2, NH], f8, isOutput=False)
    ow1sd = nc.declare_dram_parameter("ow1s", [NP, 128, 2, NH], f8, isOutput=False)
    ow1td = nc.declare_dram_parameter("ow1t", [NI, NH], bf, isOutput=False)
    ow2d = nc.declare_dram_parameter("ow2", [NP, 128, 2, NH], f8, isOutput=False)
    w3cd = nc.declare_dram_parameter("w3c", [128, FT], f32, isOutput=False)
    w3bd = nc.declare_dram_parameter("w3b", [128, FT], bf, isOutput=False)
    balld = nc.declare_dram_parameter("ball", [128, 40], f32, isOutput=False)
    ob3d = nc.declare_dram_parameter("ob3", [1, 1], f32, isOutput=False)
    outd = nc.declare_dram_parameter("out", [1, R], f32, isOutput=True)

    Relu = mybir.ActivationFunctionType.Relu
    Sigmoid = mybir.ActivationFunctionType.Sigmoid
    Identity = mybir.ActivationFunctionType.Identity
    Add = mybir.AluOpType.add
    Mult = mybir.AluOpType.mult

    # epilogue scale constants: out_next = relu(psum * k + b * s_next)
    K_M1 = S_M1 / PS_L1
    K_O1 = S_H1 / PS_L1
    K_M2 = S_M2 / (S_M1 * WS)
    K_M3 = S_S / (S_M2 * WS)
    K_O2 = 1.0 / (S_H1 * WS)   # h2 stored in true units (bf16)

    with tile.TileContext(nc) as tc:
        with (
            tc.tile_pool(name="weights", bufs=1) as wp,
            tc.tile_pool(name="summary", bufs=1) as sp,
            tc.tile_pool(name="acts", bufs=4) as ap,
            tc.tile_pool(name="tow", bufs=4) as twp,
            tc.tile_pool(name="small", bufs=1) as smp,
            tc.tile_pool(name="zwork", bufs=2) as zw,
            tc.tile_pool(name="psum", bufs=3, space="PSUM") as pp,
            tc.tile_pool(name="zpsum", bufs=2, space="PSUM") as zp,
        ):
            def load_w_split(dram, name, engs):
                tiles = []
                for i in range(NP):
                    t = wp.tile([128, 2, NH], f8, tag=f"{name}{i}",
                                name=f"{name}{i}")
                    engs[i % len(engs)].dma_start(out=t, in_=dram[i])
                    tiles.append(t)
                return tiles

            # k=0 needs only smalls + tower weights + tower data + v
            # vectors; those ride the two HW DGE queues first.  The big fp8
            # summary weights (mw1s/ow1s) are first read at k=1, so they
            # load dead last and never gate the PE start.
            tow0 = twp.tile([128, R], bf, tag="tow", name="tow")
            nc.sync.dma_start(out=tow0[0:NI, :], in_=towd[0])
            v_m = smp.tile([1, NH], bf, tag="v_m", name="v_m")
            nc.scalar.dma_start(out=v_m, in_=vmd[:])
            ball = smp.tile([128, 40], f32, tag="ball", name="ball")
            nc.scalar.dma_start(out=ball, in_=balld[:])
            mw1t = wp.tile([NI, NH], bf, tag="mw1t", name="mw1t")
            nc.sync.dma_start(out=mw1t, in_=mw1td[:])
            v_o = smp.tile([1, NH], bf, tag="v_o", name="v_o")
            nc.scalar.dma_start(out=v_o, in_=vod[:])
            nc.sync.dma_start(out=tow0[64:128, :], in_=towd[0])
            ow1t = wp.tile([128, NH], bf, tag="ow1t", name="ow1t")
            nc.gpsimd.memset(ow1t[64:128, :], 0.0)
            nc.scalar.dma_start(out=ow1t[64:128, :], in_=ow1td[:])
            ob3 = smp.tile([1, 1], f32, tag="ob3", name="ob3")
            nc.gpsimd.dma_start(out=ob3, in_=ob3d[:])
            w3c = smp.tile([128, FT], f32, tag="w3c", name="w3c")
            nc.gpsimd.dma_start(out=w3c, in_=w3cd[:])
            w3b = smp.tile([128, FT], bf, tag="w3b", name="w3b")
            nc.gpsimd.dma_start(out=w3b, in_=w3bd[:])
            mw2 = load_w_split(mw2d, "mw2", [nc.sync, nc.scalar])
            mw3 = load_w_split(mw3d, "mw3", [nc.sync, nc.scalar])
            ow2 = load_w_split(ow2d, "ow2", [nc.sync, nc.scalar])
            mw1s = load_w_split(mw1sd, "mw1s",
                                [nc.sync, nc.scalar, nc.gpsimd])
            ow1s = load_w_split(ow1sd, "ow1s",
                                [nc.sync, nc.scalar, nc.gpsimd])

            ones = smp.tile([128, 1], bf, tag="ones", name="ones")
            nc.vector.memset(ones, 1.0)
            onesrow = smp.tile([1, RB], bf, tag="onesrow", name="onesrow")
            nc.vector.memset(onesrow, 1.0)

            # --- summary double buffer: fp8 pair tiles over both row
            # blocks.  sA is never read at k=0 (step-0 summary contribution
            # is rank-1), so no initialization is needed. ---
            sA = [sp.tile([128, 2, NR, RB], f8, tag=f"sA{i}", name=f"sA{i}")
                  for i in range(NP)]
            sB = [sp.tile([128, 2, NR, RB], f8, tag=f"sB{i}", name=f"sB{i}")
                  for i in range(NP)]

            # --- product accumulators ---
            pacc = []
            for r in range(NR):
                t = smp.tile([1, RB], f32, tag=f"pacc{r}", name=f"pacc{r}")
                nc.vector.memset(t, 1.0)
                pacc.append(t)

            # bias column index per layer: 0=Mb1 1=Mb2 2=Mb3 3=Ob1 4=Ob2
            def epilogue(ot, ps, bias_l, m, k):
                """Single ACT op drains both row-block banks of one m."""
                nc.scalar.activation(ot, ps[:, :, :], Relu,
                                     bias=ball[:, ds(bias_l * 8 + m, 1)],
                                     scale=k)

            def dr_group(ps, ws, rhs, m):
                """Both row-block accumulation groups of output tile m,
                stationary weights back-to-back per contraction pair."""
                for i in range(NP):
                    for r in range(NR):
                        nc.tensor.matmul(
                            ps[:, r, :], ws[i][:, :, ts(m, 128)],
                            rhs[i][:, :, r, :],
                            start=(i == 0), stop=(i == NP - 1),
                            perf_mode=DR)

            def layer1(scur, tow_t, branches=("m", "o")):
                """Fused M/O layer 1.  Per (branch, m): 2x4 DoubleRow fp8
                matmuls over the summary pairs, closed by bf16 tower
                matmuls (M on PE rows 0-63, O on rows 64-127 so each M/O
                pair streams concurrently)."""
                m1o, h1o = [None] * FT, [None] * FT
                for m in range(FT):
                    psm = pso = None
                    if "m" in branches:
                        psm = pp.tile([128, NR, RB], f32, tag="ps",
                                      name="psm")
                        for i in range(NP):
                            for r in range(NR):
                                nc.tensor.matmul(
                                    psm[:, r, :], mw1s[i][:, :, ts(m, 128)],
                                    scur[i][:, :, r, :],
                                    start=(i == 0), stop=False,
                                    perf_mode=DR)
                    if "o" in branches:
                        pso = pp.tile([128, NR, RB], f32, tag="ps",
                                      name="pso")
                        for i in range(NP):
                            for r in range(NR):
                                nc.tensor.matmul(
                                    pso[:, r, :], ow1s[i][:, :, ts(m, 128)],
                                    scur[i][:, :, r, :],
                                    start=(i == 0), stop=False,
                                    perf_mode=DR)
                    for r in range(NR):
                        if "m" in branches:
                            nc.tensor.matmul(
                                psm[:, r, :], mw1t[:, ts(m, 128)],
                                tow_t[0:NI, ts(r, RB)],
                                start=False, stop=True)
                        if "o" in branches:
                            nc.tensor.matmul(
                                pso[:, r, :], ow1t[64:128, ts(m, 128)],
                                tow_t[64:128, ts(r, RB)],
                                start=False, stop=True)
                    if "m" in branches:
                        if m % 2 == 0:
                            mt = ap.tile([128, 2, NR, RB], f8, tag="m1",
                                         name="m1")
                            m1o[m // 2] = mt
                        epilogue(mt[:, m % 2, :, :], psm, 0, m, K_M1)
                    if "o" in branches:
                        if m % 2 == 0:
                            ht = ap.tile([128, 2, NR, RB], f8, tag="h1",
                                         name="h1")
                            h1o[m // 2] = ht
                        epilogue(ht[:, m % 2, :, :], pso, 3, m, K_O1)
                return m1o[:NP], h1o[:NP]

            def layer(rhs, ws, bias_l, k, out_mode, out_tiles=None):
                """rhs: [NP] fp8 pair tiles.  out_mode: 'pair' -> new fp8
                pair tiles, 'spair' -> write into out_tiles (summary
                pairs), 'flat' -> bf16 flat tiles (h2, both row blocks)."""
                outs = [None] * FT
                cur = None
                for m in range(FT):
                    ps = pp.tile([128, NR, RB], f32, tag="ps", name="ps")
                    dr_group(ps, ws, rhs, m)
                    if out_mode == "flat":
                        ot = ap.tile([128, R], bf, tag="h2", name="h2",
                                     bufs=8)
                        nc.scalar.activation(ot[:], ps[:, :, :], Relu,
                                             bias=ball[:, ds(bias_l * 8 + m, 1)],
                                             scale=k)
                        outs[m] = ot
                    else:
                        if out_mode == "spair":
                            cur = out_tiles[m // 2]
                        elif m % 2 == 0:
                            cur = ap.tile([128, 2, NR, RB], f8, tag="l2",
                                          name="l2")
                        epilogue(cur[:, m % 2, :, :], ps, bias_l, m, k)
                        outs[m] = cur
                if out_mode == "flat":
                    return outs
                return [outs[2 * p] for p in range(NP)]

            # Step-0 rank-1 trick: summary0 = broadcast(aggregate) is the
            # same for every row, so its layer-1 contribution is a single
            # vector v = agg @ W1s per branch -- precomputed exactly on the
            # host and shipped scaled by PS_L1 so the k=0 PSUM groups carry
            # the same scale as the generic path.
            def layer1_k0(tow_t, v_m, v_o):
                m1o, h1o = [None] * FT, [None] * FT
                for br in ("m", "o"):
                    v_sb = v_m if br == "m" else v_o
                    for m in range(FT):
                        ps = pp.tile([128, NR, RB], f32, tag="ps",
                                     name="psk0")
                        for r in range(NR):
                            nc.tensor.matmul(
                                ps[:, r, :], v_sb[0:1, ts(m, 128)],
                                onesrow[:], start=True, stop=False)
                            if br == "m":
                                nc.tensor.matmul(
                                    ps[:, r, :], mw1t[:, ts(m, 128)],
                                    tow_t[0:NI, ts(r, RB)],
                                    start=False, stop=True)
                            else:
                                nc.tensor.matmul(
                                    ps[:, r, :], ow1t[64:128, ts(m, 128)],
                                    tow_t[64:128, ts(r, RB)],
                                    start=False, stop=True)
                        if br == "m":
                            if m % 2 == 0:
                                mt = ap.tile([128, 2, NR, RB], f8,
                                             tag="m1", name="m1")
                                m1o[m // 2] = mt
                            epilogue(mt[:, m % 2, :, :], ps, 0, m, K_M1)
                        else:
                            if m % 2 == 0:
                                ht = ap.tile([128, 2, NR, RB], f8,
                                             tag="h1", name="h1")
                                h1o[m // 2] = ht
                            epilogue(ht[:, m % 2, :, :], ps, 3, m, K_O1)
                return m1o[:NP], h1o[:NP]

            def flush_zjobs(zjobs):
                for gb, r in zjobs:
                    zps = zp.tile([1, RB], f32, tag="z", name="zps")
                    nc.tensor.matmul(zps[:], ones[:], gb[:, ts(r, RB)],
                                     start=True, stop=True)
                    pr = smp.tile([1, RB], f32, tag=f"pr{r}",
                                  name=f"pr{r}")
                    nc.scalar.activation(pr[:], zps[:], Sigmoid, bias=ob3[:])
                    nc.vector.tensor_mul(pacc[r][:], pacc[r][:], pr[:])

            scur, snxt = sA, sB
            zjobs = []
            for k in range(K):
                if k == 0:
                    tow_t = tow0
                else:
                    tow_t = twp.tile([128, R], bf, tag="tow", name="tow")
                    nc.gpsimd.dma_start(out=tow_t[0:NI, :], in_=towd[k])
                    nc.gpsimd.dma_start(out=tow_t[64:128, :], in_=towd[k])

                if k == 0:
                    m1, h1 = layer1_k0(tow_t, v_m, v_o)
                elif k == K - 1:
                    # the final scan carry is discarded by the reference, so
                    # the last step's M branch (M1/M2/M3) is dead code
                    m1, h1 = layer1(scur, tow_t, branches=("o",))
                else:
                    m1, h1 = layer1(scur, tow_t)
                if k < K - 1:
                    m2 = layer(m1, mw2, 1, K_M2, "pair")
                    # previous step's output head (its DVE reduce is long
                    # done, so the sigmoid never head-of-line-blocks ACT)
                    flush_zjobs(zjobs)
                    zjobs = []
                    layer(m2, mw3, 2, K_M3, "spair", out_tiles=snxt)
                else:
                    flush_zjobs(zjobs)
                    zjobs = []
                h2 = layer(h1, ow2, 4, K_O2, "flat")
                # g = sum_i h2_i * w3_i on the DVE (per-partition scalars),
                # reduced across partitions next step by a ones-matmul.
                if k < K - 1:
                    # DVE-serial chain over both row blocks; latency is
                    # hidden by the next step's PE work
                    g = zw.tile([128, R], f32, tag="g", name="g")
                    nc.vector.tensor_scalar(
                        g[:], h2[0][:], w3c[:, ds(0, 1)], None, Mult)
                    for i in range(1, FT):
                        t = zw.tile([128, R], f32, tag="t", name="t",
                                    bufs=3)
                        nc.vector.tensor_scalar(
                            t[:], h2[i][:], w3c[:, ds(i, 1)], None, Mult)
                        nc.vector.tensor_tensor(g[:], g[:], t[:], Add)
                    gb = zw.tile([128, R], bf, tag="gb", name="gb", bufs=2)
                    nc.vector.tensor_copy(gb[:], g[:])
                    zjobs.append((gb, 0))
                    zjobs.append((gb, 1))
                else:
                    # final step: the PE is idle by now, so the whole
                    # 1024->1 reduce runs as 8 accumulating matmuls (bf16
                    # w3 columns as stationary) straight into [1, RB]
                    # PSUMs, skipping the DVE chain on the critical tail.
                    for r in range(NR):
                        zps = zp.tile([1, RB], f32, tag="z", name="zps")
                        for i in range(FT):
                            nc.tensor.matmul(
                                zps[:], w3b[:, ds(i, 1)],
                                h2[i][:, ts(r, RB)],
                                start=(i == 0), stop=(i == FT - 1))
                        pr = smp.tile([1, RB], f32, tag=f"pr{r}",
                                      name=f"pr{r}")
                        nc.scalar.activation(pr[:], zps[:], Sigmoid,
                                             bias=ob3[:])
                        nc.vector.tensor_mul(pacc[r][:], pacc[r][:],
                                             pr[:])

                scur, snxt = snxt, scur

            for r in range(NR):
                nc.sync.dma_start(out=outd[:, ts(r, RB)], in_=pacc[r][:])

    nc.finalize()
    return nc


def _get_nc():
    global _BUILT
    if _BUILT is None:
        _BUILT = _build()
    return _BUILT


def _pack_pairs(W, scale):
    """[1024, NH] f32 -> [NP, 128, 2, NH] fp8 contraction pairs."""
    Wq = (np.asarray(W, np.float32) * scale).astype(FP8)
    return np.ascontiguousarray(
        Wq.reshape(NP, 2, 128, NH).transpose(0, 2, 1, 3))


def _prep_inputs(inputs):
    f32 = np.float32
    towers = np.asarray(inputs["towers"], dtype=f32)
    agg = np.asarray(inputs["aggregate"], dtype=f32)
    MW1 = np.asarray(inputs["MW1"], dtype=f32)
    OW1 = np.asarray(inputs["OW1"], dtype=f32)

    biases = []
    for bname, s in (("Mb1", S_M1), ("Mb2", S_M2), ("Mb3", S_S),
                     ("Ob1", S_H1), ("Ob2", 1.0)):
        biases.append((np.asarray(inputs[bname], f32) * s).reshape(FT, 128).T)

    # step-0 rank-1 layer-1 contribution, exact on host (1-row matvec)
    v_m = (agg.reshape(1, NH) @ MW1[:NH]) * PS_L1
    v_o = (agg.reshape(1, NH) @ OW1[:NH]) * PS_L1

    shared = {
        "v_m": v_m.astype(BF16),
        "v_o": v_o.astype(BF16),
        "mw1s": _pack_pairs(MW1[:NH], WS),
        "mw1t": np.ascontiguousarray(MW1[NH:] * PS_L1).astype(BF16),
        "mw2": _pack_pairs(inputs["MW2"], WS),
        "mw3": _pack_pairs(inputs["MW3"], WS),
        "ow1s": _pack_pairs(OW1[:NH], WS),
        "ow1t": np.ascontiguousarray(OW1[NH:] * PS_L1).astype(BF16),
        "ow2": _pack_pairs(inputs["OW2"], WS),
        "w3c": np.ascontiguousarray(
            np.asarray(inputs["OW3"], f32).reshape(FT, 128).T),
        "w3b": np.ascontiguousarray(
            np.asarray(inputs["OW3"], f32).reshape(FT, 128).T).astype(BF16),
        "ball": np.ascontiguousarray(np.concatenate(biases, axis=1)),
        "ob3": np.asarray(inputs["Ob3"], f32).reshape(1, 1),
    }
    in_maps = []
    for c in range(N_CORES):
        tc_ = towers[c * R:(c + 1) * R]          # (R, K, NI)
        towT = np.ascontiguousarray(tc_.transpose(1, 2, 0)).astype(BF16)
        in_maps.append({"tow": towT, **shared})
    return in_maps


def _run(inputs, trace=False):
    nc = _get_nc()
    in_maps = _prep_inputs(inputs)
    res = run_bass_kernel_spmd(nc, in_maps, list(range(N_CORES)), trace=trace)
    out = np.concatenate([res.results[c]["out"][0] for c in range(N_CORES)])
    return out.astype(np.float32), res


def kernel(**inputs):
    out, _ = _run(inputs, trace=False)
    return out
